# revision 2
# baseline (speedup 1.0000x reference)
"""Trainium2 Bass kernel for nn_CkyLinear: grouped-dequant linear.

reference: W_r = ((W_q - zero) * scale).reshape(4096, 4096); out = x @ W_r.T + bias
  x     [8, 2048, 4096] f32
  W_q   [64, 262144] int32 (u8 codes)
  scale [1, 262144] f32
  zero  [1, 262144] f32
  bias  [4096] f32

Sharding: tensor-parallel over output features, 8 cores x 512 features.
Per core: dequantize its W shard on-chip (DVE) into a resident [4096, 512]
float32r weight, then stream x^T tiles and run float32r matmuls
(lhsT = x^T tile [128i, 128bs] stationary, rhs = W tile [128i, 512o] moving,
psum [128bs, 512o] accumulated over 32 k-tiles). Bias is added by DVE during
PSUM->SBUF eviction. Output shard [16384, 512] f32, host concat over features.

Group layout note: W_q[g, n] with n = h*4096 + i maps to W_r[o=g*64+h, i],
so W_r^T[i, o] for core c (o = 512c + gl*64 + h, gl in 0..8) is
(W_q[8c+gl, h*4096+i] - zero[h*4096+i]) * scale[h*4096+i]: scale/zero depend
only on (h, i) -> one shared [4096, 512] scale/zero-scale table for all cores.
"""
import sys

if "/opt/trn_rl_repo" not in sys.path:
    sys.path.insert(0, "/opt/trn_rl_repo")

import numpy as np

import concourse.bass as bass
import concourse.tile as tile
from concourse import bacc, mybir
from concourse.bass_utils import run_bass_kernel_spmd

B, S, IN_F, OUT_F, GROUP = 8, 2048, 4096, 4096, 64
BS = B * S  # 16384
N_CORES = 8
O_SHARD = OUT_F // N_CORES  # 512
KT = IN_F // 128  # 32 k-tiles
BSB = 256  # bs columns fetched per x DMA (2 matmul groups of 128)
N_BST = BS // BSB  # 64
P = 128

_CACHED_NC = None


def _build():
    nc = bacc.Bacc(trn_type="TRN2", target_bir_lowering=False, debug=False)
    f32 = mybir.dt.float32
    f32r = mybir.dt.float32r

    xt = nc.dram_tensor("xt", [IN_F, BS], f32r, kind="ExternalInput").ap()
    wq = nc.dram_tensor("wq", [IN_F, O_SHARD], mybir.dt.uint8, kind="ExternalInput").ap()
    scl = nc.dram_tensor("scl", [IN_F, O_SHARD], f32, kind="ExternalInput").ap()
    zs = nc.dram_tensor("zs", [IN_F, O_SHARD], f32, kind="ExternalInput").ap()
    bias_b = nc.dram_tensor("bias_b", [P, O_SHARD], f32, kind="ExternalInput").ap()
    out = nc.dram_tensor("out", [BS, O_SHARD], f32, kind="ExternalOutput").ap()

    xt4 = xt.rearrange("(kt p) (t b) -> p kt t b", p=P, b=BSB)
    wq3 = wq.rearrange("(kt p) o -> p kt o", p=P)
    scl3 = scl.rearrange("(kt p) o -> p kt o", p=P)
    zs3 = zs.rearrange("(kt p) o -> p kt o", p=P)
    out3 = out.rearrange("(t h b) o -> t h b o", h=BSB // P, b=P)

    with tile.TileContext(nc) as tc:
        with (
            tc.tile_pool(name="wres", bufs=1) as wres_pool,
            tc.tile_pool(name="deq", bufs=3) as deq_pool,
            tc.tile_pool(name="bias", bufs=1) as bias_pool,
            tc.tile_pool(name="xin", bufs=3) as x_pool,
            tc.tile_pool(name="psum", bufs=8, space="PSUM") as psum_pool,
            tc.tile_pool(name="oev", bufs=4) as o_pool,
        ):
            bias_sb = bias_pool.tile([P, O_SHARD], f32)
            nc.sync.dma_start(bias_sb[:], bias_b[:])

            # Dequantize W shard into resident f32 tiles (one per k-tile so
            # matmuls only wait on their own k-tile's dequant).
            w_res = []
            for k in range(KT):
                wq_t = deq_pool.tile([P, O_SHARD], mybir.dt.uint8, name="wq_t")
                sc_t = deq_pool.tile([P, O_SHARD], f32, name="sc_t")
                zs_t = deq_pool.tile([P, O_SHARD], f32, name="zs_t")
                nc.sync.dma_start(wq_t[:], wq3[:, k, :])
                nc.sync.dma_start(sc_t[:], scl3[:, k, :])
                nc.sync.dma_start(zs_t[:], zs3[:, k, :])
                w_k = wres_pool.tile([P, O_SHARD], f32r, name=f"w_{k}")
                nc.vector.tensor_mul(w_k[:], wq_t[:], sc_t[:])
                nc.vector.tensor_sub(w_k[:], w_k[:], zs_t[:])
                w_res.append(w_k)

            for t in range(N_BST):
                x_t = x_pool.tile([P, KT, BSB], f32r, name="x_t")
                nc.sync.dma_start(x_t[:], xt4[:, :, t, :])
                for h in range(BSB // P):
                    ps = psum_pool.tile([P, O_SHARD], f32, name="ps")
                    for k in range(KT):
                        nc.tensor.matmul(
                            ps[:],
                            x_t[:, k, bass.ts(h, P)],
                            w_res[k][:],
                            start=(k == 0),
                            stop=(k == KT - 1),
                        )
                    ob = o_pool.tile([P, O_SHARD], f32, name="ob")
                    nc.vector.tensor_add(ob[:], ps[:], bias_sb[:])
                    nc.sync.dma_start(out3[t, h], ob[:])
    nc.compile()
    return nc


def kernel(x, W_q, scale, zero, bias):
    global _CACHED_NC
    if _CACHED_NC is None:
        _CACHED_NC = _build()
    nc = _CACHED_NC

    x = np.asarray(x)
    W_q = np.asarray(W_q)
    scale = np.asarray(scale)
    zero = np.asarray(zero)
    bias = np.asarray(bias)

    # Host-side layout prep (sharding + transposes, no arithmetic on W codes).
    xt = np.ascontiguousarray(
        x.reshape(BS, IN_F).T.astype(np.float32, copy=False)
    )  # [IN_F, BS]
    w3 = W_q.astype(np.uint8).reshape(GROUP, GROUP, IN_F)  # [g, h, i]
    s2 = scale.astype(np.float32).reshape(GROUP, IN_F)  # [h, i]
    zs2 = (zero.astype(np.float32).reshape(GROUP, IN_F) * s2)  # [h, i]
    # shared dequant tables [i, (gl, h)] = value[h, i]
    scl_rep = np.ascontiguousarray(
        np.broadcast_to(s2.T[:, None, :], (IN_F, N_CORES, GROUP)).reshape(
            IN_F, O_SHARD
        )
    )
    zs_rep = np.ascontiguousarray(
        np.broadcast_to(zs2.T[:, None, :], (IN_F, N_CORES, GROUP)).reshape(
            IN_F, O_SHARD
        )
    )

    in_maps = []
    for c in range(N_CORES):
        wq_c = np.ascontiguousarray(
            w3[N_CORES * c : N_CORES * (c + 1)].transpose(2, 0, 1).reshape(
                IN_F, O_SHARD
            )
        )  # [i, gl*64+h]
        bias_c = bias[O_SHARD * c : O_SHARD * (c + 1)].astype(np.float32)
        bias_bc = np.ascontiguousarray(np.broadcast_to(bias_c, (P, O_SHARD)))
        in_maps.append(
            {"xt": xt, "wq": wq_c, "scl": scl_rep, "zs": zs_rep, "bias_b": bias_bc}
        )

    res = run_bass_kernel_spmd(nc, in_maps, core_ids=list(range(N_CORES)))
    out = np.concatenate([res.results[c]["out"] for c in range(N_CORES)], axis=1)
    return out.reshape(B, S, OUT_F)


# revision 3
# speedup vs baseline: 1.5127x; 1.5127x over previous
"""Trainium2 Bass kernel for nn_CkyLinear: grouped-dequant linear.

reference: W_r = ((W_q - zero) * scale).reshape(4096, 4096); out = x @ W_r.T + bias
  x     [8, 2048, 4096] f32
  W_q   [64, 262144] int32 (u8 codes)
  scale [1, 262144] f32
  zero  [1, 262144] f32
  bias  [4096] f32

Sharding: tensor-parallel over output features, 8 cores x 512 features
(column-parallel linear; x replicated, per the op's group layout the
scale/zero tables are shared by all cores).

Per core: dequantize the W shard on-chip (DVE) into a resident [4096, 512]
float32r weight, then stream x^T tiles and run float32r matmuls
(lhsT = x^T tile [128i, 128bs] stationary, rhs = W tile [128i, 512o] moving,
psum [128bs, 512o] accumulated over 32 k-tiles). Bias is added by DVE during
PSUM->SBUF eviction. Output shard [16384, 512] f32, host concat over features.

Layout notes:
- x is staged host-side as [t, p, kt, b] (t: 64 bs-tiles of 256, p: 128
  partitions = i%128, kt: 32 k-tiles, b: bs within tile) so each x-tile DMA
  reads one contiguous 32 KiB run per partition (descriptor-cheap, line-rate).
- W_q[g, n] with n = h*4096 + i maps to W_r[o=g*64+h, i]; per-core codes are
  staged as [i, gl*64+h]. scale/zero depend only on (h, i): DMA'd as [i, 64]
  tables and broadcast 8x along the free dim inside the dequant DVE ops.
- DMA is split across both HWDGE rings (sync + scalar) to overlap the x
  stream with dequant/output traffic.
"""
import sys

if "/opt/trn_rl_repo" not in sys.path:
    sys.path.insert(0, "/opt/trn_rl_repo")

import numpy as np

import concourse.bass as bass
import concourse.tile as tile
from concourse import bacc, mybir
from concourse.bass_utils import run_bass_kernel_spmd

B, S, IN_F, OUT_F, GROUP = 8, 2048, 4096, 4096, 64
BS = B * S  # 16384
N_CORES = 8
O_SHARD = OUT_F // N_CORES  # 512
KT = IN_F // 128  # 32 k-tiles
BSB = 256  # bs columns per x tile (2 matmul groups of 128)
N_BST = BS // BSB  # 64
P = 128

_CACHED_NC = None


def _build():
    nc = bacc.Bacc(trn_type="TRN2", target_bir_lowering=False, debug=False)
    f32 = mybir.dt.float32
    f32r = mybir.dt.float32r

    # x staged as [t*128, kt*256]: row t*128+p holds x[t*256 : (t+1)*256,
    # kt*128+p] runs, i.e. per tile one contiguous [32, 256] run per partition.
    xt = nc.dram_tensor("xt", [N_BST * P, KT * BSB], f32r, kind="ExternalInput").ap()
    wq = nc.dram_tensor("wq", [IN_F, O_SHARD], mybir.dt.uint8, kind="ExternalInput").ap()
    scl = nc.dram_tensor("scl", [IN_F, GROUP], f32, kind="ExternalInput").ap()
    zs = nc.dram_tensor("zs", [IN_F, GROUP], f32, kind="ExternalInput").ap()
    bias_b = nc.dram_tensor("bias_b", [P, O_SHARD], f32, kind="ExternalInput").ap()
    out = nc.dram_tensor("out", [BS, O_SHARD], f32, kind="ExternalOutput").ap()

    xt3 = xt.rearrange("(t p) f -> t p f", p=P)  # [64, 128, 8192]
    wq3 = wq.rearrange("(kt p) o -> kt p o", p=P)
    scl3 = scl.rearrange("(kt p) h -> kt p h", p=P)
    zs3 = zs.rearrange("(kt p) h -> kt p h", p=P)
    out3 = out.rearrange("(t h b) o -> t h b o", h=BSB // P, b=P)

    with tile.TileContext(nc) as tc:
        with (
            tc.tile_pool(name="wres", bufs=1) as wres_pool,
            tc.tile_pool(name="deq", bufs=3) as deq_pool,
            tc.tile_pool(name="bias", bufs=1) as bias_pool,
            tc.tile_pool(name="xin", bufs=3) as x_pool,
            tc.tile_pool(name="psum", bufs=8, space="PSUM") as psum_pool,
            tc.tile_pool(name="oev", bufs=4) as o_pool,
        ):
            bias_sb = bias_pool.tile([P, O_SHARD], f32)
            nc.scalar.dma_start(bias_sb[:], bias_b[:])

            # Dequantize W shard into resident f32r tiles (one tile per k so
            # each matmul only waits on its own k-tile's dequant).
            w_res = []
            for k in range(KT):
                wq_t = deq_pool.tile([P, O_SHARD], mybir.dt.uint8, name="wq_t")
                sc_t = deq_pool.tile([P, GROUP], f32, name="sc_t")
                zs_t = deq_pool.tile([P, GROUP], f32, name="zs_t")
                nc.scalar.dma_start(wq_t[:], wq3[k])
                nc.scalar.dma_start(sc_t[:], scl3[k])
                nc.scalar.dma_start(zs_t[:], zs3[k])
                w_k = wres_pool.tile([P, O_SHARD], f32r, name=f"w_{k}")
                w_k3 = w_k[:].rearrange("p (g h) -> p g h", h=GROUP)
                sc_b = sc_t[:, None, :].broadcast_to([P, O_SHARD // GROUP, GROUP])
                zs_b = zs_t[:, None, :].broadcast_to([P, O_SHARD // GROUP, GROUP])
                wq_t3 = wq_t[:].rearrange("p (g h) -> p g h", h=GROUP)
                nc.vector.tensor_mul(w_k3, wq_t3, sc_b)
                nc.vector.tensor_sub(w_k3, w_k3, zs_b)
                w_res.append(w_k)

            for t in range(N_BST):
                x_t = x_pool.tile([P, KT, BSB], f32r, name="x_t")
                # alternate HWDGE rings for the dominant x stream
                dma_eng = nc.sync if t % 2 == 0 else nc.scalar
                dma_eng.dma_start(
                    x_t[:], xt3[t].rearrange("p (kt b) -> p kt b", b=BSB)
                )
                for h in range(BSB // P):
                    ps = psum_pool.tile([P, O_SHARD], f32, name="ps")
                    for k in range(KT):
                        nc.tensor.matmul(
                            ps[:],
                            x_t[:, k, bass.ts(h, P)],
                            w_res[k][:],
                            start=(k == 0),
                            stop=(k == KT - 1),
                        )
                    ob = o_pool.tile([P, O_SHARD], f32, name="ob")
                    nc.vector.tensor_add(ob[:], ps[:], bias_sb[:])
                    nc.sync.dma_start(out3[t, h], ob[:])
    nc.compile()
    return nc


def kernel(x, W_q, scale, zero, bias):
    global _CACHED_NC
    if _CACHED_NC is None:
        _CACHED_NC = _build()
    nc = _CACHED_NC

    x = np.asarray(x)
    W_q = np.asarray(W_q)
    scale = np.asarray(scale)
    zero = np.asarray(zero)
    bias = np.asarray(bias)

    # Host-side layout staging (sharding + transposes, no W arithmetic).
    # x[t*256+b, kt*128+p] -> xh[t*128+p, kt*256+b]
    xh = np.ascontiguousarray(
        x.reshape(N_BST, BSB, KT, P).transpose(0, 3, 2, 1).reshape(N_BST * P, KT * BSB)
    ).astype(np.float32, copy=False)
    w3 = W_q.astype(np.uint8).reshape(GROUP, GROUP, IN_F)  # [g, h, i]
    s2 = scale.astype(np.float32).reshape(GROUP, IN_F)  # [h, i]
    zs2 = zero.astype(np.float32).reshape(GROUP, IN_F) * s2  # [h, i]
    sclT = np.ascontiguousarray(s2.T)  # [i, h]
    zsT = np.ascontiguousarray(zs2.T)  # [i, h]

    in_maps = []
    for c in range(N_CORES):
        wq_c = np.ascontiguousarray(
            w3[N_CORES * c : N_CORES * (c + 1)]
            .transpose(2, 0, 1)
            .reshape(IN_F, O_SHARD)
        )  # [i, gl*64+h]
        bias_c = bias[O_SHARD * c : O_SHARD * (c + 1)].astype(np.float32)
        bias_bc = np.ascontiguousarray(np.broadcast_to(bias_c, (P, O_SHARD)))
        in_maps.append(
            {"xt": xh, "wq": wq_c, "scl": sclT, "zs": zsT, "bias_b": bias_bc}
        )

    res = run_bass_kernel_spmd(nc, in_maps, core_ids=list(range(N_CORES)))
    out = np.concatenate([res.results[c]["out"] for c in range(N_CORES)], axis=1)
    return out.reshape(B, S, OUT_F)
